# revision 5
# baseline (speedup 1.0000x reference)
"""Trainium2 Bass kernel for nn_CritiGraph.

Math (bitwise-exact vs the fp32 reference):
  dist(c1,c2,n) = sg * (1 - e/16) * n,  sg = sign(c1)*sign(c2),
  e = frexp_exp(|c1|^|c2| + 1) = bexp(float(|c1|^|c2|) + 1.5) - 126.
  ct[t,s,c,tp] = Q[t,s,tp] + M1[t,s,tp] * R[t,s,c,tp]
  where R = sgc * (e-16) (cnc sign applied via bf16 sign-bit xor),
        M1 = -sgp * norm / 128  (pos sign + /TP/16 folded),
        Q  = (sum_tp g - g)/8,  g = cos_sta_pos.

Pipeline per t (v2 — no GPSIMD, fused ops):
  p1: z16 = cmag ^ pmag, fused with +1.5 -> f32 w1 (DVE dual-op TS) if P1_FUSED,
      else u16 xor on DVE + ACT convert.
  p3: sa(bf16) = (w1.bits >> 23) - 142   (DVE dual-op TS; replaces the old
      separate shift + catastrophic 29.5us/op GPSIMD subtract)
  p4: r16 = sa.bits ^ sign_plane         (DVE TT)
  p5: out[s,c,tp] = r*M1 + Q             (split ACT / DVE per tp)

Sharding: T=128 rows split across 8 cores (16 rows each). Inputs are full
tensors; each core receives its T-slice. Output gathered on host.
"""
import dataclasses
import numpy as np

import concourse.bass as bass
import concourse.mybir as mybir
from concourse import tile, bacc
from concourse.bass_utils import run_bass_kernel_spmd

dt = mybir.dt
Alu = mybir.AluOpType
Act = mybir.ActivationFunctionType

T, S, TP, C = 128, 128, 8, 257
NCORES = 8
TL = T // NCORES          # 16 t-rows per core
CP = 260                  # padded c per tp block: 520B -> 4B-aligned blocks
FDP = TP * CP             # 2080, tp-major free width

# which tp slices of p5 run on ACT (rest on DVE)
P5_ACT_TPS = (3, 4, 5, 6, 7)


def _rep128(ap_row):
    """DRAM row AP -> same row broadcast to 128 partitions (stride-0)."""
    return dataclasses.replace(ap_row, ap=[[0, 128]] + list(ap_row.ap)[1:])


def build_nc(*, p5_act_tps=None):
    if p5_act_tps is None:
        p5_act_tps = P5_ACT_TPS
    nc = bacc.Bacc("TRN2", target_bir_lowering=False, debug=False)

    sta_d = nc.dram_tensor("sta_loc", [TL, TP], dt.int32, kind="ExternalInput")
    pos_d = nc.dram_tensor("pos_loc", [TL, S, TP], dt.int32, kind="ExternalInput")
    cnc_d = nc.dram_tensor("cnc_loc", [TL, C, TP], dt.int32, kind="ExternalInput")
    norm_d = nc.dram_tensor("eu_norm", [TL, S], dt.float32, kind="ExternalInput")
    ct_d = nc.dram_tensor("ct", [TL, S, C, TP], dt.float32, kind="ExternalOutput")

    with tile.TileContext(nc) as tc:
        with (
            tc.tile_pool(name="const", bufs=1) as cpool,
            tc.tile_pool(name="work", bufs=3) as wpool,
            tc.tile_pool(name="dram", bufs=1, space="DRAM") as dpool,
        ):
            # ---------------- preprocessing (small) ----------------
            cncraw = cpool.tile([TL, C * TP], dt.int32)
            nc.sync.dma_start(cncraw[:], cnc_d[:].rearrange("t c p -> t (c p)"))

            # tp-major magnitude / sign planes, padded to CP per tp block,
            # stored ADJACENT in one tile so the per-t broadcast is one DMA.
            c_both = cpool.tile([TL, 2 * FDP], dt.uint16)
            nc.vector.memset(c_both[:], 0)
            c_mag16 = c_both[:, 0:FDP]
            c_sgn16 = c_both[:, FDP:2 * FDP]
            # read (c,tp) natural; write offset tp*CP + c
            cnc_r = cncraw[:].rearrange("t (c p) -> t c p", p=TP)
            cmag_w = dataclasses.replace(
                c_mag16, ap=[list(c_mag16.ap)[0], [1, C], [CP, TP]])
            csgn_w = dataclasses.replace(
                c_sgn16, ap=[list(c_sgn16.ap)[0], [1, C], [CP, TP]])
            nc.scalar.activation(cmag_w, cnc_r, Act.Abs)
            nc.vector.tensor_scalar(csgn_w, cnc_r, 0.0, 32768.0, Alu.is_lt, Alu.mult)

            # stage to DRAM for per-t replicated reads
            d_cboth = dpool.tile([TL, 2 * FDP], dt.uint16)
            nc.sync.dma_start(d_cboth[:], c_both[:])

            # pos: [s, (t,tp)]
            posraw = cpool.tile([S, TL * TP], dt.int32)
            for t in range(TL):
                nc.sync.dma_start(posraw[:, t * TP:(t + 1) * TP], pos_d[t])
            p_mag16 = cpool.tile([S, TL * TP], dt.uint16)
            nc.scalar.activation(p_mag16[:], posraw[:], Act.Abs)

            # norm[s, t] via strided DMA (4B gather, 8KB once)
            norm_sb = cpool.tile([S, TL], dt.float32)
            norm_src = dataclasses.replace(
                norm_d[:].flatten(), ap=[[1, S], [S, TL]])
            nc.sync.dma_start(norm_sb[:], norm_src)
            normB = dataclasses.replace(
                norm_sb[:], ap=[list(norm_sb[:].ap)[0], [1, TL], [0, TP]])

            # M1[s,(t,tp)] = -sgp*norm/128 (exact: +-1/128 * norm)
            sgp2 = cpool.tile([S, TL * TP], dt.float32)
            nc.vector.tensor_scalar(sgp2[:], posraw[:], 0.0, 2.0, Alu.is_lt, Alu.mult)
            sgpm = cpool.tile([S, TL * TP], dt.float32)
            nc.vector.tensor_scalar(sgpm[:], sgp2[:], 1.0, 1.0 / 128, Alu.subtract, Alu.mult)
            M1 = cpool.tile([S, TL * TP], dt.float32)
            nc.vector.tensor_tensor(
                M1[:].rearrange("s (t p) -> s t p", p=TP), sgpm[:].rearrange("s (t p) -> s t p", p=TP),
                normB, Alu.mult)

            # sta replicated to all partitions: [s, (t,tp)]
            starep = cpool.tile([S, TL * TP], dt.int32)
            sta_src = dataclasses.replace(
                sta_d[:].flatten(), ap=[[0, S], [1, TL * TP]])
            nc.sync.dma_start(starep[:], sta_src)

            # g path -> Q
            stamag = cpool.tile([S, TL * TP], dt.uint16)
            nc.scalar.activation(stamag[:], starep[:], Act.Abs)
            zg16 = cpool.tile([S, TL * TP], dt.uint16)
            nc.vector.tensor_tensor(zg16[:], stamag[:], p_mag16[:], Alu.bitwise_xor)
            wg = cpool.tile([S, TL * TP], dt.float32)
            nc.scalar.activation(wg[:], zg16[:], Act.Copy, bias=1.5, scale=1.0)
            eg32 = cpool.tile([S, TL * TP], dt.int32)
            nc.vector.tensor_scalar(eg32[:], wg[:].bitcast(dt.int32), 23, None,
                                    Alu.logical_shift_right)
            sag = cpool.tile([S, TL * TP], dt.bfloat16)
            nc.vector.tensor_scalar(sag[:], eg32[:], 142.0, None, Alu.subtract)
            sgxp = cpool.tile([S, TL * TP], dt.int32)
            nc.vector.tensor_tensor(sgxp[:], starep[:], posraw[:], Alu.bitwise_xor)
            sgx32 = cpool.tile([S, TL * TP], dt.int32)
            nc.vector.tensor_scalar(sgx32[:], sgxp[:], 16, 0x8000,
                                    Alu.logical_shift_right, Alu.bitwise_and)
            sgx16 = cpool.tile([S, TL * TP], dt.uint16)
            nc.vector.tensor_scalar(sgx16[:], sgx32[:], 1.0, None, Alu.mult)
            rg16 = cpool.tile([S, TL * TP], dt.uint16)
            nc.vector.tensor_tensor(rg16[:], sag[:].bitcast(dt.uint16), sgx16[:],
                                    Alu.bitwise_xor)
            t1 = cpool.tile([S, TL * TP], dt.float32)
            nc.vector.tensor_tensor(
                t1[:].rearrange("s (t p) -> s t p", p=TP),
                rg16[:].bitcast(dt.bfloat16).rearrange("s (t p) -> s t p", p=TP),
                normB, Alu.mult)
            t2 = cpool.tile([S, TL], dt.float32)
            nc.vector.tensor_reduce(t2[:].unsqueeze(2),
                                    t1[:].rearrange("s (t p) -> s t p", p=TP),
                                    axis=mybir.AxisListType.X, op=Alu.add)
            t2s = cpool.tile([S, TL], dt.float32)
            nc.vector.tensor_scalar(t2s[:], t2[:], 1.0 / 128, None, Alu.mult)
            Q = cpool.tile([S, TL * TP], dt.float32)
            nc.vector.scalar_tensor_tensor(
                Q[:].rearrange("s (t p) -> s t p", p=TP),
                t1[:].rearrange("s (t p) -> s t p", p=TP),
                1.0 / 128,
                t2s[:].unsqueeze(2).to_broadcast([S, TL, TP]),
                Alu.mult, Alu.subtract)

            # ---------------- main loop over t ----------------
            for t in range(TL):
                # one broadcast DMA: [mag | sign] planes for this t
                bothR = wpool.tile([S, 2 * FDP], dt.uint16, tag="bothR")
                nc.sync.dma_start(bothR[:], _rep128(d_cboth[t:t + 1, :]))
                cncR = bothR[:, 0:FDP]
                scR = bothR[:, FDP:2 * FDP]

                # p1: z = cmag ^ pmag (u16, 4x perf mode on aligned CP blocks)
                z16 = wpool.tile([S, FDP], dt.uint16, tag="z16")
                for tp in range(TP):
                    nc.vector.tensor_scalar(
                        z16[:, tp * CP:(tp + 1) * CP],
                        cncR[:, tp * CP:(tp + 1) * CP],
                        p_mag16[:, t * TP + tp: t * TP + tp + 1],
                        None, Alu.bitwise_xor)
                # p2: w1 = float(z) + 1.5 (ACT)
                w1 = wpool.tile([S, FDP], dt.float32, tag="w1")
                nc.scalar.activation(w1[:], z16[:], Act.Copy, bias=1.5, scale=1.0)

                # p3a: e32 = w1.bits >> 23; p3b: sa(bf16) = e32 - 142 (both DVE;
                # a fused (shift,sub) dual-op is rejected: bitwise+arith mix)
                e32 = wpool.tile([S, FDP], dt.int32, tag="e32")
                nc.vector.tensor_scalar(e32[:], w1[:].bitcast(dt.int32), 23, None,
                                        Alu.logical_shift_right)
                sa = wpool.tile([S, FDP], dt.bfloat16, tag="sa")
                nc.vector.tensor_scalar(sa[:], e32[:], 142.0, None, Alu.subtract)

                # r16 = sa.bits ^ sign plane
                r16 = wpool.tile([S, FDP], dt.uint16, tag="r16")
                nc.vector.tensor_tensor(r16[:], sa[:].bitcast(dt.uint16), scR,
                                        Alu.bitwise_xor)

                out_sb = wpool.tile([S, C, TP], dt.float32, tag="out")
                for tp in range(TP):
                    rsl = r16[:, tp * CP: tp * CP + C].bitcast(dt.bfloat16)
                    m1c = M1[:, t * TP + tp: t * TP + tp + 1]
                    qc = Q[:, t * TP + tp: t * TP + tp + 1]
                    if tp in p5_act_tps:
                        nc.scalar.activation(out_sb[:, :, tp], rsl, Act.Identity,
                                             bias=qc, scale=m1c)
                    else:
                        nc.vector.tensor_scalar(out_sb[:, :, tp], rsl, m1c, qc,
                                                Alu.mult, Alu.add)

                nc.sync.dma_start(ct_d[t], out_sb[:])

    nc.compile()
    return nc


_NC_CACHE = None


def kernel(sta_loc, pos_loc, cnc_loc, eu_norm):
    global _NC_CACHE
    if _NC_CACHE is None:
        _NC_CACHE = build_nc()
    nc = _NC_CACHE

    sta_loc = np.ascontiguousarray(np.asarray(sta_loc, dtype=np.int32))
    pos_loc = np.ascontiguousarray(np.asarray(pos_loc, dtype=np.int32))
    cnc_loc = np.ascontiguousarray(np.asarray(cnc_loc, dtype=np.int32))
    eu_norm = np.ascontiguousarray(np.asarray(eu_norm, dtype=np.float32))

    in_maps = []
    for c in range(NCORES):
        lo, hi = c * TL, (c + 1) * TL
        in_maps.append({
            "sta_loc": sta_loc[lo:hi],
            "pos_loc": pos_loc[lo:hi],
            "cnc_loc": cnc_loc[lo:hi],
            "eu_norm": eu_norm[lo:hi],
        })
    res = run_bass_kernel_spmd(nc, in_maps, core_ids=list(range(NCORES)))
    out = np.concatenate([r["ct"] for r in res.results], axis=0)
    return out


def run_traced(inputs, trace=True, **build_kwargs):
    """For test.py: run with NTFF tracing, return (out, BassKernelResults)."""
    global _NC_CACHE
    if build_kwargs:
        nc = build_nc(**build_kwargs)
    else:
        if _NC_CACHE is None:
            _NC_CACHE = build_nc()
        nc = _NC_CACHE
    in_maps = []
    for c in range(NCORES):
        lo, hi = c * TL, (c + 1) * TL
        in_maps.append({k: np.ascontiguousarray(v[lo:hi]) for k, v in inputs.items()})
    res = run_bass_kernel_spmd(nc, in_maps, core_ids=list(range(NCORES)), trace=trace)
    out = np.concatenate([r["ct"] for r in res.results], axis=0)
    return out, res
